# revision 1
# baseline (speedup 1.0000x reference)
"""Deformable-conv block kernel for Trainium2 (8 NeuronCores, batch-parallel).

Per core (one batch sample):
  conv1(3x3) -> BN1+ReLU -> offset conv(3x3, 18ch) -> deformable conv(3x3)
  -> BN2+ReLU

Layout strategy:
  - Feature maps channel-major [128c, H*W] for all convs (matmul rhs with
    shifted access patterns over a zero-padded buffer).
  - Post-BN1 map Y additionally PE-transposed to position-major
    [(H+4)*(W+4), 128c] in DRAM, the source for the bilinear corner gather
    (nc.gpsimd.dma_gather; one descriptor fetches an x-adjacent corner pair).
  - Bilinear weights applied position-major via per-partition-scalar
    tensor_scalar / scalar_tensor_tensor chains, PE-transposed back to
    channel-major, then the 9-tap einsum accumulates in PSUM.
  - Out-of-bounds samples are handled exactly by the 2-wide zero pad of the
    gather source (coordinates clamped to [-2, 65] address the pad).
"""

import numpy as np
import ml_dtypes

B, C, H, W = 8, 128, 64, 64
K = 9
EPS = 1e-5
PW = 66          # conv pad layout width (64 + 2*1)
GP = 68          # gather pad layout width (64 + 2*2)
GROWS = GP * GP + GP   # bottom pair of the last row may index one row past 68*68
GBUF_ROWS = 4736  # 37*128 allocated DRAM rows for the gather source
NHALF = 2048     # positions per processing half
MAGIC = 8388608.0  # 2^23, fp32 round-to-nearest-int magic

bf16 = ml_dtypes.bfloat16

_LERP2 = None


def _register_lerp2():
    """Register a custom fused DVE op: out = Src0*C0 + Src1*C1 (per-partition
    scalars). Halves the bilinear-weighting instruction count."""
    global _LERP2
    if _LERP2 is not None:
        return _LERP2
    import concourse.dve_ops as D
    from concourse.dve_spec import Spec, Src0, Src1, C0, C1, lower, _has_src1
    from concourse.dve_uop import DveOpSpec

    name = "LERP2_ANT"
    if name in D._SUB_OPCODE_FOR_NAME:
        _LERP2 = next(op for op in D.OPS if op.name == name)
        return _LERP2

    spec = Spec(
        body=Src0 * C0 + Src1 * C1,
        reference=lambda in0, in1, s0, s1, imm2:
            in0.astype(np.float32) * s0 + in1.astype(np.float32) * s1,
    )
    opcode = max(D._SUB_OPCODE_FOR_NAME.values()) + 1
    assert opcode < 0x20
    shas = {}
    for ver in ("v3",):
        s = DveOpSpec(name=name, opcode=opcode, uops=lower(spec, ver=ver),
                      rd1_en=_has_src1(spec))
        shas[ver] = s.sha(ver)
    op = D.DveOp(name, spec, subdim=False, uops_sha=shas)
    D._SUB_OPCODE_FOR_NAME[name] = opcode
    D.OPS.append(op)
    D.CUSTOM_DVE_SPECS[name] = spec
    _LERP2 = op
    return op


def _prep_inputs(x, conv1_w, conv1_b, bn1_g, bn1_b, bn1_m, bn1_v,
                 off_w, off_b, dconv_w, dconv_b, bn2_g, bn2_b, bn2_m, bn2_v):
    """Host-side layout prep. Returns per-core input maps."""
    s1 = (bn1_g / np.sqrt(bn1_v + EPS)).astype(np.float32)
    b1 = (bn1_b + (conv1_b - bn1_m) * s1).astype(np.float32)
    s2 = (bn2_g / np.sqrt(bn2_v + EPS)).astype(np.float32)
    b2 = (bn2_b + (dconv_b - bn2_m) * s2).astype(np.float32)

    # lhsT layouts [c, k, o]
    w1sb = np.ascontiguousarray(
        conv1_w.transpose(1, 2, 3, 0).reshape(C, K, C)).astype(bf16)
    woffsb = np.ascontiguousarray(
        off_w.transpose(1, 2, 3, 0).reshape(C, K, 18)).astype(bf16)
    wdsb = np.ascontiguousarray(
        (dconv_w * s2[:, None, None, None]).transpose(1, 2, 3, 0)
        .reshape(C, K, C)).astype(bf16)

    # base sample coordinates, position-major [pp, j, k]
    p = np.arange(4096)
    ppj = p.reshape(32, 128)                      # [j, pp]
    yy = (ppj // 64).astype(np.float32)
    xx = (ppj % 64).astype(np.float32)
    ky = (np.arange(K) // 3).astype(np.float32)
    kx = (np.arange(K) % 3).astype(np.float32)
    cy = (yy.T[:, :, None] + ky[None, None, :] - 1.0).astype(np.float32)
    cx = (xx.T[:, :, None] + kx[None, None, :] - 1.0).astype(np.float32)

    ident_bf = np.eye(C, dtype=np.float32).astype(bf16)
    ident_f32 = np.eye(C, dtype=np.float32)

    shared = {
        "w1sb": w1sb, "woffsb": woffsb, "wdsb": wdsb,
        "bn1s": s1.reshape(C, 1), "bn1b": b1.reshape(C, 1),
        "b2": b2.reshape(C, 1),
        "offbias": off_b.reshape(18, 1).astype(np.float32),
        "cy": cy, "cx": cx, "identbf": ident_bf, "identf": ident_f32,
    }

    per_core = []
    for b in range(B):
        xpad = np.zeros((C, PW, PW), dtype=bf16)
        xpad[:, 1:65, 1:65] = x[b].astype(bf16)
        m = dict(shared)
        m["xpad"] = xpad.reshape(C, PW * PW)
        per_core.append(m)
    return per_core


def build_kernel_body(ctx, tc, outs, ins):
    import concourse.bass as bass
    import concourse.mybir as mybir
    from concourse.mybir import AluOpType as alu
    from concourse.mybir import ActivationFunctionType as act

    lerp2 = _register_lerp2()
    nc = tc.nc
    f32 = mybir.dt.float32
    bfl = mybir.dt.bfloat16
    i16 = mybir.dt.int16
    i32 = mybir.dt.int32

    out_d = outs["out"]

    const = ctx.enter_context(tc.tile_pool(name="const", bufs=1))
    feat = ctx.enter_context(tc.tile_pool(name="feat", bufs=1))
    dram = ctx.enter_context(tc.tile_pool(name="dram", bufs=1, space="DRAM"))
    coord = ctx.enter_context(tc.tile_pool(name="coord", bufs=1))

    # ---- constants ----
    def cload(name, shape, dt):
        t = const.tile(list(shape), dt, tag=name)
        nc.sync.dma_start(t[:], ins[name][:])
        return t

    w1 = cload("w1sb", (C, K, C), bfl)
    woff = cload("woffsb", (C, K, 18), bfl)
    wd = cload("wdsb", (C, K, C), bfl)
    bn1s = cload("bn1s", (C, 1), f32)
    bn1b = cload("bn1b", (C, 1), f32)
    b2 = cload("b2", (C, 1), f32)
    offb = cload("offbias", (18, 1), f32)
    cy = cload("cy", (C, 32, K), f32)
    cx = cload("cx", (C, 32, K), f32)
    identbf = cload("identbf", (C, C), bfl)
    identf = cload("identf", (C, C), f32)

    # +8 slack elements: conv rhs runs read a couple of junk columns past
    # the last row (results discarded)
    xpad = feat.tile([C, PW * PW + 8], bfl)
    nc.vector.memset(xpad[:, PW * PW:], 0.0)
    nc.sync.dma_start(xpad[:, 0:PW * PW], ins["xpad"][:])

    ypad = feat.tile([C, PW * PW + 8], bfl)
    nc.vector.memset(ypad[:], 0.0)
    yflat = feat.tile([C, 4096], bfl)

    ytd = dram.tile([GBUF_ROWS, C], bfl)

    # zero the gather-source DRAM buffer with one DMA from a zeroed tile
    zt = feat.tile([C, C], bfl)
    nc.vector.memset(zt[:], 0.0)
    nc.sync.dma_start(
        ytd[:].rearrange("(a p) c -> p a c", p=C),
        zt[:].unsqueeze(1).broadcast_to((C, GBUF_ROWS // C, C)))

    # ---- stage A: conv1 + BN1 + ReLU -> ypad + yflat ----
    # matmul rhs must be a single contiguous free dim: stream 4-row runs of
    # the padded image (4*66=264 cols incl 2 junk pad cols per row; junk
    # columns are not evacuated).
    with tc.tile_pool(name="psA", bufs=3, space="PSUM") as psA:
        for r0 in range(0, 64, 4):
            ps = psA.tile([C, 4 * PW], f32, tag="psA")
            for k in range(K):
                dy, dx = k // 3, k % 3
                st = (r0 + dy) * PW + dx
                nc.tensor.matmul(ps[:], w1[:, k, :], xpad[:, st: st + 4 * PW],
                                 start=(k == 0), stop=(k == K - 1))
            pv = ps[:].rearrange("c (a b) -> c a b", a=4)[:, :, 0:64]
            nc.scalar.activation(
                yflat[:, r0 * 64: (r0 + 4) * 64].rearrange(
                    "c (a b) -> c a b", a=4),
                pv, act.Relu, bias=bn1b[:], scale=bn1s[:])
            nc.scalar.activation(
                ypad[:, 0:PW * PW].rearrange("c (a b) -> c a b", a=PW)[
                    :, 1 + r0: 5 + r0, 1:65],
                pv, act.Relu, bias=bn1b[:], scale=bn1s[:])

    # ---- stage B: PE-transpose Y to position-major, DMA to DRAM ----
    with tc.tile_pool(name="psB", bufs=2, space="PSUM") as psB, \
         tc.tile_pool(name="stgB", bufs=2) as stgB:
        for g in range(8):
            ps = psB.tile([C, 4, C], bfl, tag="psB")
            for t in range(4):
                y0 = (g * 4 + t) * 2
                nc.tensor.transpose(ps[:, t, :],
                                    yflat[:, y0 * 64: y0 * 64 + 128],
                                    identbf[:])
            stg = stgB.tile([C, 4, C], bfl, tag="stgB")
            nc.scalar.copy(stg[:], ps[:])
            # partitions pp=(yy,x); dest row (8g + 2t + yy + 2)*68 + x + 2
            for t in range(4):
                dst = bass.AP(ytd[:].tensor,
                              ((8 * g + 2 * t + 2) * GP + 2) * C,
                              [[GP * C, 2], [C, 64], [1, C]])
                nc.sync.dma_start(dst, stg[:, t: t + 1, :])

    # ---- stage C: offset conv -> off_sb [18, 4096] f32 ----
    off_sb = coord.tile([18, 4096], f32)
    with tc.tile_pool(name="psC", bufs=3, space="PSUM") as psC:
        for r0 in range(0, 64, 4):
            ps = psC.tile([18, 4 * PW], f32, tag="psC")
            for k in range(K):
                dy, dx = k // 3, k % 3
                st = (r0 + dy) * PW + dx
                nc.tensor.matmul(ps[:], woff[:, k, :],
                                 ypad[:, st: st + 4 * PW],
                                 start=(k == 0), stop=(k == K - 1))
            pv = ps[:].rearrange("p (a b) -> p a b", a=4)[:, :, 0:64]
            nc.scalar.activation(
                off_sb[:, r0 * 64: (r0 + 4) * 64].rearrange(
                    "p (a b) -> p a b", a=4),
                pv, act.Identity, bias=offb[:], scale=1.0)

    # ---- stage D: transpose offsets to position-major [128, 32, 18] ----
    offT = coord.tile([C, 32, 18], f32)
    with tc.tile_pool(name="psD", bufs=2, space="PSUM") as psD:
        for g in range(8):
            ps = psD.tile([C, 4, 18], f32, tag="psD")
            for t in range(4):
                j = g * 4 + t
                nc.tensor.transpose(ps[:, t: t + 1, :],
                                    off_sb[:, j * 128: (j + 1) * 128],
                                    identf[0:18, 0:18])
            nc.vector.tensor_copy(offT[:, g * 4: g * 4 + 4, :], ps[:])

    # ---- stage E: bilinear coordinates, weights, gather indices ----
    def coord_axis(off_slice, cbase, ax):
        pc = coord.tile([C, 32, K], f32, tag=f"pc{ax}")
        nc.vector.tensor_add(pc[:], off_slice, cbase)
        r = coord.tile([C, 32, K], f32, tag=f"r{ax}")
        nc.vector.tensor_scalar(r[:], pc[:], MAGIC, MAGIC, alu.add,
                                alu.subtract)
        gtm = coord.tile([C, 32, K], f32, tag=f"gt{ax}")
        nc.vector.tensor_tensor(gtm[:], r[:], pc[:], alu.is_gt)
        fl = coord.tile([C, 32, K], f32, tag=f"fl{ax}")
        nc.vector.tensor_sub(fl[:], r[:], gtm[:])
        fr = coord.tile([C, 32, K], f32, tag=f"fr{ax}")
        nc.vector.tensor_sub(fr[:], pc[:], fl[:])
        flc = coord.tile([C, 32, K], f32, tag=f"flc{ax}")
        nc.vector.tensor_scalar(flc[:], fl[:], -2.0, 65.0, alu.max, alu.min)
        return flc, fr

    y0c, fy = coord_axis(offT[:, :, 0:18:2], cy[:], "y")
    x0c, fx = coord_axis(offT[:, :, 1:18:2], cx[:], "x")

    gy = coord.tile([C, 32, K], f32)
    nc.vector.tensor_scalar(gy[:], fy[:], -1.0, 1.0, alu.mult, alu.add)
    gx = coord.tile([C, 32, K], f32)
    nc.vector.tensor_scalar(gx[:], fx[:], -1.0, 1.0, alu.mult, alu.add)

    # corner pair row indices: top = (y0c+2)*68 + (x0c+2); bot = top + 68
    # laid out [pp, k, tb, jj] so each (k, tb) slice is a contiguous
    # [128, 32] block for the wrap transposes below
    idxf = coord.tile([C, K, 2, 32], f32)
    vtop = idxf[:, :, 0, :].rearrange("p k j -> p j k")
    vbot = idxf[:, :, 1, :].rearrange("p k j -> p j k")
    t1 = coord.tile([C, 32, K], f32)
    nc.vector.tensor_scalar(t1[:], y0c[:], 68.0, 138.0, alu.mult, alu.add)
    nc.vector.tensor_add(vtop, t1[:], x0c[:])
    nc.vector.tensor_scalar_add(vbot, vtop, 68.0)

    # ---- stage F: wrap indices into dma_gather's 16-partition layout ----
    # gather sequence i -> partition i%16, column i//16. With i = jl*128+pp
    # (pp = b*16+q): idxs[q, (h,k,tb)-block, jl*8+b] = idxf[b*16+q, k, tb,
    # h*16+jl]. Done per (k, tb) via transpose / reorder / 8 sliced
    # transposes / strided evac.
    idxs16 = coord.tile([C, 2, K, 2, 128], i16)
    with tc.tile_pool(name="psW", bufs=4, space="PSUM") as psW, \
         tc.tile_pool(name="wrapv", bufs=2) as wrapv:
        for k in range(K):
            for tb in range(2):
                p1 = psW.tile([32, C], f32, tag="p1")
                nc.tensor.transpose(p1[:], idxf[:, k, tb, :], identf[:])
                v = wrapv.tile([32, C], f32, tag="v")
                nc.vector.tensor_copy(
                    v[:].rearrange("j (q b) -> j q b", b=8),
                    p1[:].rearrange("j (b q) -> j q b", b=8))
                p2 = psW.tile([16, 8, 32], f32, tag="p2")
                for b in range(8):
                    nc.tensor.transpose(p2[:, b, :], v[:, b::8],
                                        identf[0:32, 0:32])
                nc.vector.tensor_copy(
                    idxs16[0:16, :, k, tb, :].rearrange(
                        "q h (jl b) -> q b h jl", b=8),
                    p2[:].rearrange("q b (h jl) -> q b h jl", h=2))
    # replicate across the 8 Q7 core partition groups
    for r in range(1, 8):
        nc.sync.dma_start(idxs16[16 * r:16 * (r + 1)], idxs16[0:16])

    # gather source: row pitch 128 elems, element = 2 consecutive rows
    gsrc = bass.AP(ytd[:].tensor, 0, [[C, GROWS], [1, 2 * C]])

    # ---- stages G/H: gather, bilinear-weight, transpose, einsum ----
    with tc.tile_pool(name="gat", bufs=3) as gat, \
         tc.tile_pool(name="spool", bufs=2) as spool, \
         tc.tile_pool(name="ssum", bufs=4) as ssum, \
         tc.tile_pool(name="psT", bufs=2, space="PSUM") as psT, \
         tc.tile_pool(name="psO", bufs=1, space="PSUM") as psO, \
         tc.tile_pool(name="outp", bufs=2) as outp:
        for h in range(2):
            po = psO.tile([C, NHALF], f32, tag="psO")
            for k in range(K):
                gtop = gat.tile([C, 16, 2 * C], bfl, tag="gtop")
                nc.gpsimd.dma_gather(gtop[:], gsrc, idxs16[:, h, k, 0, :],
                                     NHALF, NHALF, 2 * C, elem_step=C,
                                     single_packet=False)
                gbot = gat.tile([C, 16, 2 * C], bfl, tag="gbot")
                nc.gpsimd.dma_gather(gbot[:], gsrc, idxs16[:, h, k, 1, :],
                                     NHALF, NHALF, 2 * C, elem_step=C,
                                     single_packet=False)
                s_tap = spool.tile([C, NHALF], bfl, tag="stap")
                for jq in range(4):
                    ps = psT.tile([C, 4, C], bfl, tag="psT")
                    for t in range(4):
                        j = jq * 4 + t
                        jj = h * 16 + j
                        # vertical lerp on the x-pair, then horizontal lerp
                        tv = ssum.tile([C, 2 * C], bfl, tag="tv")
                        nc.vector._custom_dve(
                            lerp2, out=tv[:], in0=gtop[:, j, :],
                            in1=gbot[:, j, :], s0=gy[:, jj, k: k + 1],
                            s1=fy[:, jj, k: k + 1])
                        t0 = ssum.tile([C, C], bfl, tag="t0")
                        nc.vector._custom_dve(
                            lerp2, out=t0[:], in0=tv[:, 0:C],
                            in1=tv[:, C:2 * C], s0=gx[:, jj, k: k + 1],
                            s1=fx[:, jj, k: k + 1])
                        nc.tensor.transpose(ps[:, t, :], t0[:], identbf[:])
                    nc.scalar.copy(
                        s_tap[:, jq * 512:(jq + 1) * 512].rearrange(
                            "p (a b) -> p a b", a=4), ps[:])
                for n4 in range(4):
                    nc.tensor.matmul(
                        po[:, n4 * 512:(n4 + 1) * 512], wd[:, k, :],
                        s_tap[:, n4 * 512:(n4 + 1) * 512],
                        start=(k == 0), stop=(k == K - 1))
            ob = outp.tile([C, NHALF], f32, tag="ob")
            nc.scalar.activation(ob[:], po[:], act.Relu, bias=b2[:],
                                 scale=1.0)
            nc.sync.dma_start(out_d[:, h * NHALF:(h + 1) * NHALF], ob[:])


def _build_program():
    from contextlib import ExitStack
    import concourse.bacc as bacc
    import concourse.tile as tile
    import concourse.mybir as mybir

    f32 = mybir.dt.float32
    bfl = mybir.dt.bfloat16

    nc = bacc.Bacc("TRN2", target_bir_lowering=False, debug=False,
                   enable_asserts=False, num_devices=B)

    ins = {}

    def din(name, shape, dt):
        ins[name] = nc.dram_tensor(name, list(shape), dt,
                                   kind="ExternalInput").ap()

    din("xpad", (C, PW * PW), bfl)
    din("w1sb", (C, K, C), bfl)
    din("woffsb", (C, K, 18), bfl)
    din("wdsb", (C, K, C), bfl)
    din("bn1s", (C, 1), f32)
    din("bn1b", (C, 1), f32)
    din("b2", (C, 1), f32)
    din("offbias", (18, 1), f32)
    din("cy", (C, 32, K), f32)
    din("cx", (C, 32, K), f32)
    din("identbf", (C, C), bfl)
    din("identf", (C, C), f32)

    outs = {"out": nc.dram_tensor("out", [C, H * W], f32,
                                  kind="ExternalOutput").ap()}

    with tile.TileContext(nc) as tc:
        with ExitStack() as ctx:
            build_kernel_body(ctx, tc, outs, ins)
    nc.compile()
    return nc


_cached_nc = None
last_results = None


def kernel(**inputs) -> np.ndarray:
    global _cached_nc, last_results
    from concourse import bass_utils

    per_core = _prep_inputs(**{k: np.asarray(v) for k, v in inputs.items()})
    if _cached_nc is None:
        _cached_nc = _build_program()
    nc = _cached_nc

    res = bass_utils.run_bass_kernel_spmd(nc, per_core,
                                          core_ids=list(range(B)))
    last_results = res
    out = np.stack([res.results[b]["out"].reshape(C, H, W) for b in range(B)])
    return out.astype(np.float32)


if __name__ == "__main__":
    import reference

    inputs = {k: np.asarray(v) for k, v in reference.setup_inputs().items()}
    got = kernel(**inputs)
    print("kernel output", got.shape, got.dtype)

